# revision 8
# baseline (speedup 1.0000x reference)
"""FP64->FP32 bit-circuit converter for Trainium2 (8 NeuronCores), packed I/O.

The end-to-end cost of kernel() is dominated by host<->device transport over
the axon tunnel (~50-150 MB/s, ~140 ms fixed latency per fetch), not device
execution.  Strategy (pure data parallel over the batch, 131072 rows/core):

  host:   pack the (B, 64) {0,1}-float input into 2 int32 words per row
          (8 MB instead of 256 MB) with a multithreaded XLA-CPU jit;
  device: run the whole conversion as ~36 fused int32 ALU ops per row
          (shift/and/or/xor/add/compare) on each core's [128 x 1024] layout,
          emitting the literal IEEE fp32 bit pattern as one int32 per row
          (4 MB back instead of 128 MB);
  host:   expand the words back into the (B, 32) float bit matrix.

Bit layout (MSB-first, matching the column order of the reference):
  hi = row bits 0..31  (bit0=sign at bit31, bits1..11=exp, bits12..31=mant0..19)
  lo = row bits 32..63 (mant bits 20..51)
Output word = sign<<31 | exp8<<23 | mant23 -- exactly the fp32 bit pattern,
whose MSB-first bit expansion equals the 32 output columns.

The Bass kernel is compiled and first executed via
bass_utils.run_bass_kernel_spmd (during warm-up, which also cross-checks the
fast path against it); steady-state calls reuse one cached jit executor to
avoid per-call retracing, with the donated output buffers created on-device
so no zero pages cross the tunnel.  Warm-up starts in a background thread at
import so axon connection + neuronxcc compile overlap harness setup.
"""
import threading
import numpy as np
import jax
import jax.numpy as jnp
from jax.sharding import Mesh, PartitionSpec, NamedSharding
from jax.experimental.shard_map import shard_map

from concourse import bacc, bass2jax, mybir
from concourse.tile import TileContext
from concourse.bass_utils import run_bass_kernel_spmd

AOT = mybir.AluOpType
I32 = mybir.dt.int32

B = 1_048_576
N_CORES = 8
B_CORE = B // N_CORES          # 131072
P = 128                        # SBUF partitions
NI = B_CORE // P               # 1024 rows per partition
D_IN = 2                       # packed words per row
D_OUT = 1


def _build():
    nc = bacc.Bacc("TRN2")
    x = nc.dram_tensor("x", [B_CORE, D_IN], I32, kind="ExternalInput")
    y = nc.dram_tensor("y", [B_CORE, D_OUT], I32, kind="ExternalOutput")

    x_r = x.ap().rearrange("(p n) d -> p (n d)", p=P)   # [128, NI*2]
    y_r = y.ap().rearrange("(p n) d -> p (n d)", p=P)   # [128, NI]

    NF = NI // 2  # two supertiles: overlap in-DMA / compute / out-DMA

    with TileContext(nc) as tc:
        with (
            tc.tile_pool(name="io", bufs=2) as io,
            tc.tile_pool(name="sc", bufs=2) as sc,
        ):
            for st in range(NI // NF):
                off = st * NF
                xin = io.tile([P, NF * D_IN], I32, tag="xin", name="xin")
                nc.sync.dma_start(xin[:, :], x_r[:, off * D_IN:(off + NF) * D_IN])
                xv = xin[:, :].rearrange("p (n d) -> p n d", d=D_IN)
                hi = xv[:, :, 0]
                lo = xv[:, :, 1]

                def T(tag):
                    t = sc.tile([P, NF], I32, tag=tag, name=tag)
                    return t[:, :]

                # All int32 ALU ops on vector (DVE): bitwise int32 is
                # DVE-only, and Pool-engine int ops measured ~14us each
                # (Q7 overhead + cross-engine sync), 3x worse overall.
                V = G = nc.vector

                # field extraction
                E = T("E")          # 11-bit biased fp64 exponent
                V.tensor_scalar(E, hi, 20, 0x7FF,
                                AOT.logical_shift_right, AOT.bitwise_and)
                Mhi = T("Mhi")
                V.tensor_scalar(Mhi, hi, 0xFFFFF, 3,
                                AOT.bitwise_and, AOT.logical_shift_left)
                Mlo = T("Mlo")
                V.tensor_scalar(Mlo, lo, 29, 7,
                                AOT.logical_shift_right, AOT.bitwise_and)
                M23 = T("M23")      # top 23 mantissa bits as an int
                V.tensor_tensor(M23, Mhi, Mlo, AOT.bitwise_or)
                R = T("R")          # round bit (mant bit 23)
                V.tensor_scalar(R, lo, 28, 1,
                                AOT.logical_shift_right, AOT.bitwise_and)
                sval = T("sval")    # sticky field (mant bits 24..51)
                V.tensor_scalar(sval, lo, 0x0FFFFFFF, None, AOT.bitwise_and)
                S = T("S")
                G.tensor_scalar(S, sval, 1, None, AOT.min)
                # round-to-nearest-even: round_up = R & (S | lsb)
                L = T("L")
                V.tensor_scalar(L, M23, 1, None, AOT.bitwise_and)
                SL = T("SL")
                V.tensor_tensor(SL, S, L, AOT.bitwise_or)
                ru = T("ru")
                V.tensor_tensor(ru, R, SL, AOT.bitwise_and)
                Mr = T("Mr")
                V.tensor_tensor(Mr, M23, ru, AOT.add)
                c_m = T("c_m")      # mantissa carry into the exponent
                V.tensor_scalar(c_m, Mr, 23, None, AOT.logical_shift_right)
                mant_f = T("mant_f")
                V.tensor_scalar(mant_f, Mr, 0x7FFFFF, None, AOT.bitwise_and)
                # rebias: newE = (E - 896) + c_m
                newE = T("newE")
                V.scalar_tensor_tensor(newE, E, -896, c_m, AOT.add, AOT.add)
                nsh = T("nsh")
                V.tensor_scalar(nsh, newE, 23, None, AOT.logical_shift_left)
                body = T("body")
                V.tensor_tensor(body, nsh, mant_f, AOT.bitwise_or)
                # specials (feeders, off the critical chain)
                over = T("over")
                G.tensor_scalar(over, E, 1151, None, AOT.is_ge)
                under = T("under")
                G.tensor_scalar(under, E, 897, None, AOT.is_lt)
                lml = T("lml")      # mant bits 23..51
                V.tensor_scalar(lml, lo, 0x1FFFFFFF, None, AOT.bitwise_and)
                manyv = T("manyv")
                V.tensor_tensor(manyv, M23, lml, AOT.bitwise_or)
                eq2047 = T("eq2047")
                G.tensor_scalar(eq2047, E, 2047, None, AOT.is_equal)
                many = T("many")
                G.tensor_scalar(many, manyv, 1, None, AOT.min)
                nan = T("nan")
                V.tensor_tensor(nan, eq2047, many, AOT.bitwise_and)
                om = T("om")
                G.tensor_scalar(om, over, 1, None, AOT.subtract)
                um = T("um")
                G.tensor_scalar(um, under, 1, None, AOT.subtract)
                nm = T("nm")
                G.tensor_scalar(nm, nan, 1, None, AOT.subtract)
                sb = T("sb")
                V.tensor_scalar(sb, hi, 31, 31,
                                AOT.logical_shift_right, AOT.logical_shift_left)
                # body1 = over ? 0x7F800000 : body   (xor/and with NOT-mask)
                x1 = T("x1")
                V.tensor_scalar(x1, body, 0x7F800000, None, AOT.bitwise_xor)
                x2 = T("x2")
                V.tensor_tensor(x2, x1, om, AOT.bitwise_and)
                body1 = T("body1")
                V.tensor_scalar(body1, x2, 0x7F800000, None, AOT.bitwise_xor)
                # body2 = under ? 0 : body1
                body2 = T("body2")
                V.tensor_tensor(body2, body1, um, AOT.bitwise_and)
                # body3 = nan ? 0x7FC00000 : body2
                x3 = T("x3")
                V.tensor_scalar(x3, body2, 0x7FC00000, None, AOT.bitwise_xor)
                x4 = T("x4")
                V.tensor_tensor(x4, x3, nm, AOT.bitwise_and)
                body3 = T("body3")
                V.tensor_scalar(body3, x4, 0x7FC00000, None, AOT.bitwise_xor)
                yt = io.tile([P, NF], I32, tag="yt", name="yt")
                V.tensor_tensor(yt[:, :], body3, sb, AOT.bitwise_or)
                nc.sync.dma_start(y_r[:, off:off + NF], yt[:, :])

    nc.compile()
    return nc


# ---------------- host-side pack / unpack (XLA CPU, multithreaded) ----------
_PACK_W = (np.uint32(1) << np.arange(31, -1, -1, dtype=np.uint32)).astype(np.int32)


def _pack_cpu(xf):
    # {0.,1.} float bits, MSB-first -> int32 words; int32 add-wrap == OR here
    xi = xf.astype(jnp.int32).reshape(-1, D_IN, 32)
    return (xi * _PACK_W[None, None, :]).sum(axis=-1, dtype=jnp.int32)


def _unpack_cpu(w):
    sh = jnp.arange(31, -1, -1, dtype=jnp.int32)
    bits = jnp.right_shift(w.reshape(-1, 1).view(jnp.uint32),
                           sh.view(jnp.uint32)[None, :]) & jnp.uint32(1)
    return bits.astype(jnp.float32)


def _pack_input_np(x: np.ndarray) -> np.ndarray:
    xp = np.packbits(x != 0, axis=-1)
    return xp.view(np.dtype(">u4")).astype(np.uint32).view(np.int32)


def _unpack_output_np(w: np.ndarray) -> np.ndarray:
    wbe = w.view(np.uint32).astype(np.dtype(">u4"))
    bits = np.unpackbits(wbe.view(np.uint8).reshape(-1, 4), axis=-1)
    return bits.astype(np.float32)


# ---------------- cached executor ----------------
_STATE: dict = {}
_LOCK = threading.Lock()


def _prepare_locked():
    if "ready" in _STATE or "failed" in _STATE:
        return
    try:
        nc = _build()
        _STATE["nc"] = nc

        # official path first: compile + run the Bass kernel via
        # run_bass_kernel_spmd (dummy input); also warms devices + NEFF.
        dummy = np.zeros((B_CORE, D_IN), np.int32)
        in_maps = [{"x": dummy} for _ in range(N_CORES)]
        res = run_bass_kernel_spmd(nc, in_maps, core_ids=list(range(N_CORES)))
        w_official = np.concatenate([r["y"] for r in res.results], axis=0)

        pack_jit = jax.jit(_pack_cpu, backend="cpu")
        unpack_jit = jax.jit(_unpack_cpu, backend="cpu")

        bass2jax.install_neuronx_cc_hook()
        pn = nc.partition_id_tensor.name if nc.partition_id_tensor else None
        in_names, out_names, out_avals = [], [], []
        for alloc in nc.m.functions[0].allocations:
            if not isinstance(alloc, mybir.MemoryLocationSet):
                continue
            name = alloc.memorylocations[0].name
            if alloc.kind == "ExternalInput":
                if name != pn:
                    in_names.append(name)
            elif alloc.kind == "ExternalOutput":
                out_names.append(name)
                out_avals.append(jax.core.ShapedArray(
                    tuple(alloc.tensor_shape), mybir.dt.np(alloc.dtype)))
        n_params, n_outs = len(in_names), len(out_avals)
        in_names_all = in_names + out_names + ([pn] if pn else [])
        donate = tuple(range(n_params, n_params + n_outs))

        def _body(*args):
            operands = list(args)
            if pn is not None:
                operands.append(bass2jax.partition_id_tensor())
            return tuple(bass2jax._bass_exec_p.bind(
                *operands, out_avals=tuple(out_avals),
                in_names=tuple(in_names_all), out_names=tuple(out_names),
                lowering_input_output_aliases=(),
                sim_require_finite=True, sim_require_nnan=True, nc=nc))

        devices = jax.devices()[:N_CORES]
        mesh = Mesh(np.asarray(devices), ("core",))
        spec = PartitionSpec("core")
        shd = NamedSharding(mesh, spec)
        sharded = jax.jit(
            shard_map(_body, mesh=mesh, in_specs=(spec,) * (n_params + n_outs),
                      out_specs=(spec,) * n_outs, check_rep=False),
            donate_argnums=donate, keep_unused=True)
        g_out = (N_CORES * out_avals[0].shape[0], *out_avals[0].shape[1:])
        zeros_jit = jax.jit(lambda: jnp.zeros(g_out, out_avals[0].dtype),
                            out_shardings=shd)

        # warm-compile + cross-check the fast path against the official run
        xg = np.zeros((B, D_IN), np.int32)
        out = sharded(xg, zeros_jit())
        w_fast = np.asarray(out[0])
        assert np.array_equal(w_fast, w_official), "fast path mismatch"
        # warm the host pack/unpack jits too
        pack_jit(np.zeros((4096, 64), np.float32))
        unpack_jit(np.zeros((4096, 1), np.int32))

        _STATE.update(dict(pack_jit=pack_jit, unpack_jit=unpack_jit,
                           sharded=sharded, zeros_jit=zeros_jit, ready=True))
    except Exception as e:  # fall back to the plain spmd path per call
        _STATE["failed"] = repr(e)
        if "nc" not in _STATE:
            _STATE["nc"] = _build()


def _prepare():
    with _LOCK:
        _prepare_locked()


def _get_nc():
    _prepare()
    return _STATE["nc"]


_WARM = threading.Thread(target=_prepare, daemon=True)
_WARM.start()


def kernel(fp64_pulse: np.ndarray) -> np.ndarray:
    x = np.asarray(fp64_pulse)
    assert x.shape == (B, 64)
    _prepare()
    if "ready" in _STATE:
        try:
            zeros = _STATE["zeros_jit"]()                # async, on-device
            xw = np.asarray(_STATE["pack_jit"](x))       # (B, 2) int32
            out = _STATE["sharded"](xw, zeros)
            w = np.asarray(out[0])                       # (B, 1) int32
            return np.asarray(_STATE["unpack_jit"](w))
        except Exception:
            pass  # transient failure: serve this call via the plain path
    # fallback: plain official path with numpy pack/unpack
    nc = _STATE["nc"]
    xw = _pack_input_np(x)
    in_maps = [{"x": xw[c * B_CORE:(c + 1) * B_CORE]} for c in range(N_CORES)]
    res = run_bass_kernel_spmd(nc, in_maps, core_ids=list(range(N_CORES)))
    w = np.concatenate([r["y"] for r in res.results], axis=0)
    return _unpack_output_np(w)
